# revision 34
# baseline (speedup 1.0000x reference)
"""DiffKMeansMultiClass loss on 8 Trainium2 NeuronCores.

Strategy: group samples by class on the host (pure permutation + padding)
so each core computes, per class, distances of its shard to that class's
64 centroids only. Host precomputes normalized xn (fp16) and |xn|^2, so
the device does, per class pair (2 classes packed into 128 PSUM
partitions = 2x64 centroids; class A's slot-j sample occupies partition
rows 0:64 and class B's slot-j sample rows 64:128 via zero-padded
stationary weights):

  d2   = -2 xn.mu + (|xn|^2 + |mu|^2 + BIG*invalid)   PE (2 mu chunks fp16
         + one 2-row mini chunk injecting the additive terms)
  L    = Ln(d2)                                        ACT (fp32)
  s    = Exp(0.5 L) = sqrt(d2)                         ACT (fp16)
  e    = Exp(-3.125 s + 3.125 shift_c)                 ACT (bf16, per-class
         shift keeps the exponent in fp32/bf16 range; legal because the
         softmax ratio v/Z cancels any per-class constant)
  t    = s * e                                         DVE (bf16)
  Z(c) = sum_k e ; B(c) = sum_k c2_k e ; A(c) = sum_k c1_k s e
       = three weighted partition reductions fused into 2 PE matmuls
         (stationary [1_A,1_B,c2_A,c2_B,0,0] over e then
          [0,0,0,0,c1_A,c1_B] over t accumulating into one [6,448] PSUM)
  out  = per-sample (A - B)/Z and the segment mean happen on the host,
         where c1 = -6.25/tau, c2 = ln tau.

No transposes, no DVE reductions: per-sample softmax normalization
reduces over k which lives on PSUM partitions, so the PE's ones/weighted
matmuls do all reductions.
"""

import os
import numpy as np

N, D, C, K = 131072, 256, 20, 64
NCORES = 8
WIN = 448
P = C // 2
DIST_SCALE = 100.0 / 16.0  # 100/sqrt(256) = 6.25
CLUSTER_TEMP = 0.5
SIG_TEMP = 2.0
SIG_MAX = 100.0
RESET_THR = 0.5
BIG = 30000.0  # invalid-centroid d2 penalty (fp16-safe; exp(-3.125*sqrt) = 0)
TEMP = CLUSTER_TEMP * DIST_SCALE  # 3.125

_CACHE = {}


def _build_program(cap, patch_tables=True):
    import concourse.tile as tile
    from concourse import bacc, mybir

    f32 = mybir.dt.float32
    f16 = mybir.dt.float16
    bf16 = mybir.dt.bfloat16
    f8 = mybir.dt.float8e4
    nw = max(2, -(-cap // 512))
    WIN = cap // nw           # one matmul window (<=512 fp32 PSUM cols)
    assert cap == nw * WIN and WIN <= 512
    nr = C * cap              # rows per core
    cap2 = 2 * cap

    nc = bacc.Bacc("TRN2", target_bir_lowering=False, debug=False)
    xt = nc.dram_tensor("xt", [2, 128, nr], f8, kind="ExternalInput")
    x2r = nc.dram_tensor("x2r", [2, P, cap], f16, kind="ExternalInput")
    wm = nc.dram_tensor("wm", [128, C, 2, 128], f8, kind="ExternalInput")
    wmini = nc.dram_tensor("wmini", [2, P, 128], f16, kind="ExternalInput")
    zst = nc.dram_tensor("zst", [128, P, 2 * nw, 6 * nw], bf16,
                         kind="ExternalInput")
    bias = nc.dram_tensor("bias", [128, P, 2], f32, kind="ExternalInput")
    wout = nc.dram_tensor("wout", [P, 6 * nw, WIN], f32, kind="ExternalOutput")

    Exp = mybir.ActivationFunctionType.Exp
    Ln = mybir.ActivationFunctionType.Ln
    Alu = mybir.AluOpType

    with tile.TileContext(nc) as tc:
        with (
            tc.tile_pool(name="const", bufs=1) as const,
            tc.tile_pool(name="xtp", bufs=4) as xtp,
            tc.tile_pool(name="lp", bufs=2) as lp,
            tc.tile_pool(name="sp", bufs=2) as sp,
            tc.tile_pool(name="ep", bufs=3) as ep,
            tc.tile_pool(name="tp", bufs=3) as tp,
            tc.tile_pool(name="psd", bufs=6, space="PSUM") as psd,
            tc.tile_pool(name="psz", bufs=2, space="PSUM") as psz,
        ):
            wsb = const.tile([128, C, 2, 128], f8)
            nc.sync.dma_start(wsb[:], wm[:])
            msb = const.tile([2, P, 128], f16)
            nc.sync.dma_start(msb[:], wmini[:])
            zsb = const.tile([128, P, 2 * nw, 6 * nw], bf16)
            nc.sync.dma_start(zsb[:], zst[:])
            bsb = const.tile([128, P, 2], f32)
            nc.sync.dma_start(bsb[:], bias[:])

            # dep-free dummy activation so the 1.3us act-table load runs
            # during the initial DMA fill instead of before the first Ln
            dum = const.tile([1, 1], f32)
            nc.vector.memset(dum[:], 0.0)
            dum2 = const.tile([1, 1], f32)
            nc.scalar.activation(dum2[:], dum[:], Exp)

            deferred = []

            for p in range(P):
                cA, cB = 2 * p, 2 * p + 1
                base = cA * cap
                # one D-half per DMA (fully contiguous rows), split across
                # the two hardware DGE queues (SP + gpsimd) in parallel
                xh = xtp.tile([128, 2, cap2], f8, tag="xh")
                nc.sync.dma_start(xh[:, 0, :], xt[0, :, base:base + cap2])
                nc.gpsimd.dma_start(xh[:, 1, :], xt[1, :, base:base + cap2])
                x2t = xtp.tile([2, cap], f16, tag="x2t")
                (nc.gpsimd if p % 2 else nc.sync).dma_start(
                    x2t[:], x2r[:, p, :])

                # window-major so each window's PSUM group closes early and
                # Ln can start while later windows are still on the PE;
                # mu chunks contract all 256 dims in one fp8 DoubleRow pass
                DR = mybir.MatmulPerfMode.DoubleRow
                L = lp.tile([128, cap], f32, tag="L")
                for w in range(nw):
                    ps = psd.tile([128, WIN], f32, tag="ps",
                                  name=f"ps_{p}_{w}")
                    sA = slice(w * WIN, (w + 1) * WIN)
                    sB = slice(cap + w * WIN, cap + (w + 1) * WIN)
                    nc.tensor.matmul(ps[:], wsb[:, cA, :, :], xh[:, :, sA],
                                     perf_mode=DR, start=True, stop=False)
                    nc.tensor.matmul(ps[:], wsb[:, cB, :, :], xh[:, :, sB],
                                     perf_mode=DR, start=False, stop=False)
                    # rank-2 chunk adds |xn|^2 for both class halves
                    nc.tensor.matmul(ps[:], msb[:, p, :], x2t[:, sA],
                                     start=False, stop=True)
                    # |mu|^2 + invalid penalty enters via the Ln bias
                    nc.scalar.activation(L[:, sA], ps[:], Ln,
                                         bias=bsb[:, p, 0:1])
                s = sp.tile([128, cap], f16, tag="s")
                nc.scalar.activation(s[:], L[:], Exp, scale=0.5)
                e = ep.tile([128, cap], bf16, tag="e")
                nc.scalar.activation(e[:], s[:], Exp, bias=bsb[:, p, 1:2],
                                     scale=-TEMP)
                t = tp.tile([128, cap], bf16, tag="t")
                nc.vector.tensor_tensor(t[:], s[:], e[:], op=Alu.mult)

                def zv_stage(p=p, e=e, t=t):
                    # all 2*nw reduction matmuls accumulate into one PSUM
                    # bank: rows 6w..6w+6 belong to window w
                    zps = psz.tile([6 * nw, WIN], f32, tag="zps",
                                   name=f"zps_{p}")
                    nmm = 2 * nw
                    for w in range(nw):
                        sl = slice(w * WIN, (w + 1) * WIN)
                        nc.tensor.matmul(zps[:], zsb[:, p, 2 * w, :], e[:, sl],
                                         start=(w == 0), stop=False)
                        nc.tensor.matmul(zps[:], zsb[:, p, 2 * w + 1, :],
                                         t[:, sl],
                                         start=False, stop=(w == nw - 1))
                    zout = tp.tile([6 * nw, WIN], f32, tag="zout",
                                   name=f"zout_{p}")
                    nc.vector.tensor_copy(zout[:], zps[:])
                    (nc.gpsimd if p % 2 else nc.sync).dma_start(
                        wout[p], zout[:])

                deferred.append(zv_stage)
                # emit reductions two pairs late so the in-order PE queue
                # never waits on ACT/DVE output
                if len(deferred) > 2:
                    deferred.pop(0)()
            while deferred:
                deferred.pop(0)()

    import concourse.bacc as bacc_mod
    from concourse import hw_specs
    orig_tables = hw_specs.get_activation_tables
    want = {Ln, Exp}

    def only_cover(arch):
        full = orig_tables(arch)
        if not any(want <= s for s in full.values()):
            return full
        chosen = next(n for n, s in full.items() if want <= s)
        return {n: (s if n == chosen else set()) for n, s in full.items()}

    if patch_tables:
        bacc_mod.get_activation_tables = only_cover
    try:
        nc.finalize()
    finally:
        bacc_mod.get_activation_tables = orig_tables
    return nc


def _host_prep(data, labels, mu, exp_temp, norm_med, norm_std,
               running_assignment, running_batchsize):
    labels = np.asarray(labels).astype(np.int64)
    data = np.asarray(data, dtype=np.float32)
    mu = np.asarray(mu, dtype=np.float32)

    idx_by_class = [np.flatnonzero(labels == c) for c in range(C)]
    per_core_counts = np.zeros((C, NCORES), dtype=np.int64)
    per_core_idx = [[None] * NCORES for _ in range(C)]
    maxcnt = 1
    for c in range(C):
        splits = np.array_split(idx_by_class[c], NCORES)
        for r in range(NCORES):
            per_core_idx[c][r] = splits[r]
            per_core_counts[c, r] = len(splits[r])
            maxcnt = max(maxcnt, len(splits[r]))

    nw = max(2, -(-maxcnt // 512))
    win = -(-maxcnt // nw)
    win = (win + 15) // 16 * 16  # window multiple of 16
    cap = nw * win
    nr = C * cap

    import ml_dtypes
    f8 = ml_dtypes.float8_e4m3
    a = 1.0 / np.asarray(norm_std, dtype=np.float32)
    b = -np.asarray(norm_med, dtype=np.float32) * a
    xn8 = (data * a[None, :] + b[None, :]).astype(f8)
    x2 = (xn8.astype(np.float32) ** 2).sum(axis=1)  # [N] fp32

    xts, x2rs = [], []
    for r in range(NCORES):
        xc = np.zeros((nr, D), dtype=f8)
        x2c = np.zeros(nr, dtype=np.float32)
        for c in range(C):
            idx = per_core_idx[c][r]
            if len(idx):
                xc[c * cap:c * cap + len(idx)] = xn8[idx]
                x2c[c * cap:c * cap + len(idx)] = x2[idx]
        xts.append(np.ascontiguousarray(xc.T).reshape(2, 128, nr))
        # [2, P, cap]: row 0 = even-class |xn|^2, row 1 = odd-class
        x2rs.append(np.ascontiguousarray(
            x2c.astype(np.float16).reshape(P, 2, cap).transpose(1, 0, 2)))

    mu8 = mu.astype(f8)
    mu8f = mu8.astype(np.float32)
    m2 = (mu8f ** 2).sum(axis=2)  # [C, K]
    thr = np.asarray(running_batchsize, np.float32) / K * RESET_THR
    valid = np.asarray(running_assignment, np.float32) > thr[:, None]
    m2pen = (m2 + BIG * (~valid)).astype(np.float32)

    # zero-padded stationaries: even classes drive PSUM partitions 0:64,
    # odd classes 64:128
    wm = np.zeros((128, C, 2, 128), dtype=f8)
    wmini = np.zeros((2, P, 128), dtype=np.float16)
    for c in range(C):
        h = (c % 2) * K
        wm[:, c, 0, h:h + K] = (-2.0 * mu8f[c, :, :128]).T.astype(f8)
        wm[:, c, 1, h:h + K] = (-2.0 * mu8f[c, :, 128:]).T.astype(f8)
        wmini[c % 2, c // 2, h:h + K] = 1.0

    et = np.asarray(exp_temp, dtype=np.float32)
    tau = 1.0 / (1.0 + np.exp(-et / SIG_TEMP)) * SIG_MAX + 1.0 / SIG_MAX
    c1 = (-DIST_SCALE / tau).astype(np.float32)  # [C, K]
    c2 = np.log(tau).astype(np.float32)

    zst = np.zeros((128, P, 2 * nw, 6 * nw), dtype=ml_dtypes.bfloat16)
    for p in range(P):
        for w in range(nw):
            o = 6 * w
            zst[:K, p, 2 * w, o + 0] = 1.0
            zst[K:, p, 2 * w, o + 1] = 1.0
            zst[:K, p, 2 * w, o + 2] = c2[2 * p]
            zst[K:, p, 2 * w, o + 3] = c2[2 * p + 1]
            zst[:K, p, 2 * w + 1, o + 4] = c1[2 * p]
            zst[K:, p, 2 * w + 1, o + 5] = c1[2 * p + 1]

    shift = np.sqrt(np.median(x2) + np.median(m2, axis=1))  # [C]
    bias = np.zeros((128, P, 2), dtype=np.float32)
    for p in range(P):
        bias[:K, p, 0] = m2pen[2 * p]
        bias[K:, p, 0] = m2pen[2 * p + 1]
        bias[:K, p, 1] = TEMP * shift[2 * p]
        bias[K:, p, 1] = TEMP * shift[2 * p + 1]

    in_maps = [
        {"xt": xts[r], "x2r": x2rs[r], "wm": wm, "wmini": wmini,
         "zst": zst, "bias": bias}
        for r in range(NCORES)
    ]
    meta = {"cap": cap, "nw": nw, "counts": per_core_counts}
    return in_maps, meta


def _gather(results, meta):
    cap = meta["cap"]
    WINR = cap // meta["nw"]
    counts = meta["counts"]  # [C, NCORES]
    total = np.float64(0.0)
    for c in range(C):
        cnt_c = counts[c].sum()
        if cnt_c == 0:
            continue
        p, h = c // 2, c % 2
        seg = np.float64(0.0)
        for r in range(NCORES):
            w = results[r]["wout"].reshape(P, -1, 6, WINR)  # [P, nw, 6, WIN]
            Z = w[p, :, 0 + h, :].reshape(-1).astype(np.float64)
            B = w[p, :, 2 + h, :].reshape(-1).astype(np.float64)
            A = w[p, :, 4 + h, :].reshape(-1).astype(np.float64)
            n = counts[c, r]
            seg += -np.sum((A[:n] - B[:n]) / Z[:n])
        total += seg / cnt_c
    return np.float32(total)


def kernel(**inputs) -> np.ndarray:
    from concourse import bass_utils

    in_maps, meta = _host_prep(**inputs)
    cap = meta["cap"]
    patch_tables = bool(int(os.environ.get("KERNEL_PATCH_TABLES", "1")))
    key = (cap, patch_tables)
    if key not in _CACHE:
        _CACHE[key] = _build_program(cap, patch_tables)
    nc = _CACHE[key]

    trace = bool(int(os.environ.get("KERNEL_TRACE", "0")))
    kwargs = {}
    if trace:
        kwargs["tmpdir"] = os.environ.get("KERNEL_TRACE_DIR") or None
    res = bass_utils.run_bass_kernel_spmd(
        nc, in_maps, core_ids=list(range(NCORES)), trace=trace, **kwargs)
    if trace and res.exec_time_ns is not None:
        print(f"HW exec time: {res.exec_time_ns} ns")
    return _gather(res.results, meta)


# revision 37
# speedup vs baseline: 1.0140x; 1.0140x over previous
"""DiffKMeansMultiClass loss on 8 Trainium2 NeuronCores.

Strategy: group samples by class on the host (pure permutation + padding)
so each core computes, per class, distances of its shard to that class's
64 centroids only. Host precomputes normalized xn (fp16) and |xn|^2, so
the device does, per class pair (2 classes packed into 128 PSUM
partitions = 2x64 centroids; class A's slot-j sample occupies partition
rows 0:64 and class B's slot-j sample rows 64:128 via zero-padded
stationary weights):

  d2   = -2 xn.mu + (|xn|^2 + |mu|^2 + BIG*invalid)   PE (2 mu chunks fp16
         + one 2-row mini chunk injecting the additive terms)
  L    = Ln(d2)                                        ACT (fp32)
  s    = Exp(0.5 L) = sqrt(d2)                         ACT (fp16)
  e    = Exp(-3.125 s + 3.125 shift_c)                 ACT (bf16, per-class
         shift keeps the exponent in fp32/bf16 range; legal because the
         softmax ratio v/Z cancels any per-class constant)
  t    = s * e                                         DVE (bf16)
  Z(c) = sum_k e ; B(c) = sum_k c2_k e ; A(c) = sum_k c1_k s e
       = three weighted partition reductions fused into 2 PE matmuls
         (stationary [1_A,1_B,c2_A,c2_B,0,0] over e then
          [0,0,0,0,c1_A,c1_B] over t accumulating into one [6,448] PSUM)
  out  = per-sample (A - B)/Z and the segment mean happen on the host,
         where c1 = -6.25/tau, c2 = ln tau.

No transposes, no DVE reductions: per-sample softmax normalization
reduces over k which lives on PSUM partitions, so the PE's ones/weighted
matmuls do all reductions.
"""

import os
import numpy as np

N, D, C, K = 131072, 256, 20, 64
NCORES = 8
WIN = 448
P = C // 2
DIST_SCALE = 100.0 / 16.0  # 100/sqrt(256) = 6.25
CLUSTER_TEMP = 0.5
SIG_TEMP = 2.0
SIG_MAX = 100.0
RESET_THR = 0.5
BIG = 30000.0  # invalid-centroid d2 penalty (fp16-safe; exp(-3.125*sqrt) = 0)
TEMP = CLUSTER_TEMP * DIST_SCALE  # 3.125

_CACHE = {}


def _build_program(cap, patch_tables=True):
    import concourse.tile as tile
    from concourse import bacc, mybir

    f32 = mybir.dt.float32
    f16 = mybir.dt.float16
    bf16 = mybir.dt.bfloat16
    f8 = mybir.dt.float8e4
    nw = max(2, -(-cap // 512))
    WIN = cap // nw           # one matmul window (<=512 fp32 PSUM cols)
    assert cap == nw * WIN and WIN <= 512
    nr = C * cap              # rows per core
    cap2 = 2 * cap

    nc = bacc.Bacc("TRN2", target_bir_lowering=False, debug=False)
    xt = nc.dram_tensor("xt", [2, 128, nr], f8, kind="ExternalInput")
    x2r = nc.dram_tensor("x2r", [2, P, cap], f16, kind="ExternalInput")
    wm = nc.dram_tensor("wm", [128, C, 2, 128], f8, kind="ExternalInput")
    wmini = nc.dram_tensor("wmini", [2, P, 128], f16, kind="ExternalInput")
    zst = nc.dram_tensor("zst", [128, P, 2 * nw, 6 * nw], bf16,
                         kind="ExternalInput")
    bias = nc.dram_tensor("bias", [128, P, 2], f32, kind="ExternalInput")
    wout = nc.dram_tensor("wout", [P, 6 * nw, WIN], f32, kind="ExternalOutput")

    Exp = mybir.ActivationFunctionType.Exp
    Ln = mybir.ActivationFunctionType.Ln
    Alu = mybir.AluOpType

    with tile.TileContext(nc) as tc:
        with (
            tc.tile_pool(name="const", bufs=1) as const,
            tc.tile_pool(name="xtp", bufs=4) as xtp,
            tc.tile_pool(name="lp", bufs=2) as lp,
            tc.tile_pool(name="sp", bufs=2) as sp,
            tc.tile_pool(name="ep", bufs=3) as ep,
            tc.tile_pool(name="tp", bufs=3) as tp,
            tc.tile_pool(name="psd", bufs=6, space="PSUM") as psd,
            tc.tile_pool(name="psz", bufs=2, space="PSUM") as psz,
        ):
            wsb = const.tile([128, C, 2, 128], f8)
            nc.sync.dma_start(wsb[:], wm[:])
            msb = const.tile([2, P, 128], f16)
            nc.sync.dma_start(msb[:], wmini[:])
            zsb = const.tile([128, P, 2 * nw, 6 * nw], bf16)
            nc.sync.dma_start(zsb[:], zst[:])
            bsb = const.tile([128, P, 2], f32)
            nc.sync.dma_start(bsb[:], bias[:])

            # tiny dummy activation reading an early const so the 1.3us
            # act-table load (queued just before it, with no waits of its
            # own) runs during the initial DMA fill instead of before the
            # first Ln
            dum2 = const.tile([1, 1], f32)
            nc.scalar.activation(dum2[:], bsb[0:1, 0, 0:1], Exp)

            deferred = []

            for p in range(P):
                cA, cB = 2 * p, 2 * p + 1
                base = cA * cap
                # one D-half per DMA (fully contiguous rows), split across
                # the two hardware DGE queues (SP + gpsimd) in parallel
                xh = xtp.tile([128, 2, cap2], f8, tag="xh")
                nc.sync.dma_start(xh[:, 0, :], xt[0, :, base:base + cap2])
                nc.gpsimd.dma_start(xh[:, 1, :], xt[1, :, base:base + cap2])
                x2t = xtp.tile([2, cap], f16, tag="x2t")
                (nc.gpsimd if p % 2 else nc.sync).dma_start(
                    x2t[:], x2r[:, p, :])

                # window-major so each window's PSUM group closes early and
                # Ln can start while later windows are still on the PE;
                # mu chunks contract all 256 dims in one fp8 DoubleRow pass
                DR = mybir.MatmulPerfMode.DoubleRow
                L = lp.tile([128, cap], f32, tag="L")
                for w in range(nw):
                    ps = psd.tile([128, WIN], f32, tag="ps",
                                  name=f"ps_{p}_{w}")
                    sA = slice(w * WIN, (w + 1) * WIN)
                    sB = slice(cap + w * WIN, cap + (w + 1) * WIN)
                    nc.tensor.matmul(ps[:], wsb[:, cA, :, :], xh[:, :, sA],
                                     perf_mode=DR, start=True, stop=False)
                    nc.tensor.matmul(ps[:], wsb[:, cB, :, :], xh[:, :, sB],
                                     perf_mode=DR, start=False, stop=False)
                    # rank-2 chunk adds |xn|^2 for both class halves
                    nc.tensor.matmul(ps[:], msb[:, p, :], x2t[:, sA],
                                     start=False, stop=True)
                    # |mu|^2 + invalid penalty enters via the Ln bias
                    nc.scalar.activation(L[:, sA], ps[:], Ln,
                                         bias=bsb[:, p, 0:1])
                s = sp.tile([128, cap], f16, tag="s")
                e = ep.tile([128, cap], bf16, tag="e")
                t = tp.tile([128, cap], bf16, tag="t")
                # window-granular on the last pair to shorten the drain
                for sl in ([slice(0, cap)] if p < P - 1 else
                           [slice(w * WIN, (w + 1) * WIN) for w in range(nw)]):
                    nc.scalar.activation(s[:, sl], L[:, sl], Exp, scale=0.5)
                    nc.scalar.activation(e[:, sl], s[:, sl], Exp,
                                         bias=bsb[:, p, 1:2], scale=-TEMP)
                    nc.vector.tensor_tensor(t[:, sl], s[:, sl], e[:, sl],
                                            op=Alu.mult)

                def zv_stage(p=p, e=e, t=t):
                    # all 2*nw reduction matmuls accumulate into one PSUM
                    # bank: rows 6w..6w+6 belong to window w
                    zps = psz.tile([6 * nw, WIN], f32, tag="zps",
                                   name=f"zps_{p}")
                    nmm = 2 * nw
                    for w in range(nw):
                        sl = slice(w * WIN, (w + 1) * WIN)
                        nc.tensor.matmul(zps[:], zsb[:, p, 2 * w, :], e[:, sl],
                                         start=(w == 0), stop=False)
                        nc.tensor.matmul(zps[:], zsb[:, p, 2 * w + 1, :],
                                         t[:, sl],
                                         start=False, stop=(w == nw - 1))
                    zout = tp.tile([6 * nw, WIN], f32, tag="zout",
                                   name=f"zout_{p}")
                    nc.vector.tensor_copy(zout[:], zps[:])
                    (nc.gpsimd if p % 2 else nc.sync).dma_start(
                        wout[p], zout[:])

                deferred.append(zv_stage)
                # emit reductions two pairs late so the in-order PE queue
                # never waits on ACT/DVE output
                if len(deferred) > 2:
                    deferred.pop(0)()
            while deferred:
                deferred.pop(0)()

    import concourse.bacc as bacc_mod
    from concourse import hw_specs
    orig_tables = hw_specs.get_activation_tables
    want = {Ln, Exp}

    def only_cover(arch):
        full = orig_tables(arch)
        if not any(want <= s for s in full.values()):
            return full
        chosen = next(n for n, s in full.items() if want <= s)
        return {n: (s if n == chosen else set()) for n, s in full.items()}

    if patch_tables:
        bacc_mod.get_activation_tables = only_cover
    try:
        nc.finalize()
    finally:
        bacc_mod.get_activation_tables = orig_tables
    return nc


def _host_prep(data, labels, mu, exp_temp, norm_med, norm_std,
               running_assignment, running_batchsize):
    labels = np.asarray(labels).astype(np.int64)
    data = np.asarray(data, dtype=np.float32)
    mu = np.asarray(mu, dtype=np.float32)

    idx_by_class = [np.flatnonzero(labels == c) for c in range(C)]
    per_core_counts = np.zeros((C, NCORES), dtype=np.int64)
    per_core_idx = [[None] * NCORES for _ in range(C)]
    maxcnt = 1
    for c in range(C):
        splits = np.array_split(idx_by_class[c], NCORES)
        for r in range(NCORES):
            per_core_idx[c][r] = splits[r]
            per_core_counts[c, r] = len(splits[r])
            maxcnt = max(maxcnt, len(splits[r]))

    nw = max(2, -(-maxcnt // 512))
    win = -(-maxcnt // nw)
    win = (win + 7) // 8 * 8  # window multiple of 8
    cap = nw * win
    nr = C * cap

    import ml_dtypes
    f8 = ml_dtypes.float8_e4m3
    a = 1.0 / np.asarray(norm_std, dtype=np.float32)
    b = -np.asarray(norm_med, dtype=np.float32) * a
    xn8 = (data * a[None, :] + b[None, :]).astype(f8)
    x2 = (xn8.astype(np.float32) ** 2).sum(axis=1)  # [N] fp32

    xts, x2rs = [], []
    for r in range(NCORES):
        xc = np.zeros((nr, D), dtype=f8)
        x2c = np.zeros(nr, dtype=np.float32)
        for c in range(C):
            idx = per_core_idx[c][r]
            if len(idx):
                xc[c * cap:c * cap + len(idx)] = xn8[idx]
                x2c[c * cap:c * cap + len(idx)] = x2[idx]
        xts.append(np.ascontiguousarray(xc.T).reshape(2, 128, nr))
        # [2, P, cap]: row 0 = even-class |xn|^2, row 1 = odd-class
        x2rs.append(np.ascontiguousarray(
            x2c.astype(np.float16).reshape(P, 2, cap).transpose(1, 0, 2)))

    mu8 = mu.astype(f8)
    mu8f = mu8.astype(np.float32)
    m2 = (mu8f ** 2).sum(axis=2)  # [C, K]
    thr = np.asarray(running_batchsize, np.float32) / K * RESET_THR
    valid = np.asarray(running_assignment, np.float32) > thr[:, None]
    m2pen = (m2 + BIG * (~valid)).astype(np.float32)

    # zero-padded stationaries: even classes drive PSUM partitions 0:64,
    # odd classes 64:128
    wm = np.zeros((128, C, 2, 128), dtype=f8)
    wmini = np.zeros((2, P, 128), dtype=np.float16)
    for c in range(C):
        h = (c % 2) * K
        wm[:, c, 0, h:h + K] = (-2.0 * mu8f[c, :, :128]).T.astype(f8)
        wm[:, c, 1, h:h + K] = (-2.0 * mu8f[c, :, 128:]).T.astype(f8)
        wmini[c % 2, c // 2, h:h + K] = 1.0

    et = np.asarray(exp_temp, dtype=np.float32)
    tau = 1.0 / (1.0 + np.exp(-et / SIG_TEMP)) * SIG_MAX + 1.0 / SIG_MAX
    c1 = (-DIST_SCALE / tau).astype(np.float32)  # [C, K]
    c2 = np.log(tau).astype(np.float32)

    zst = np.zeros((128, P, 2 * nw, 6 * nw), dtype=ml_dtypes.bfloat16)
    for p in range(P):
        for w in range(nw):
            o = 6 * w
            zst[:K, p, 2 * w, o + 0] = 1.0
            zst[K:, p, 2 * w, o + 1] = 1.0
            zst[:K, p, 2 * w, o + 2] = c2[2 * p]
            zst[K:, p, 2 * w, o + 3] = c2[2 * p + 1]
            zst[:K, p, 2 * w + 1, o + 4] = c1[2 * p]
            zst[K:, p, 2 * w + 1, o + 5] = c1[2 * p + 1]

    shift = np.sqrt(np.median(x2) + np.median(m2, axis=1))  # [C]
    bias = np.zeros((128, P, 2), dtype=np.float32)
    for p in range(P):
        bias[:K, p, 0] = m2pen[2 * p]
        bias[K:, p, 0] = m2pen[2 * p + 1]
        bias[:K, p, 1] = TEMP * shift[2 * p]
        bias[K:, p, 1] = TEMP * shift[2 * p + 1]

    in_maps = [
        {"xt": xts[r], "x2r": x2rs[r], "wm": wm, "wmini": wmini,
         "zst": zst, "bias": bias}
        for r in range(NCORES)
    ]
    meta = {"cap": cap, "nw": nw, "counts": per_core_counts}
    return in_maps, meta


def _gather(results, meta):
    cap = meta["cap"]
    WINR = cap // meta["nw"]
    counts = meta["counts"]  # [C, NCORES]
    total = np.float64(0.0)
    for c in range(C):
        cnt_c = counts[c].sum()
        if cnt_c == 0:
            continue
        p, h = c // 2, c % 2
        seg = np.float64(0.0)
        for r in range(NCORES):
            w = results[r]["wout"].reshape(P, -1, 6, WINR)  # [P, nw, 6, WIN]
            Z = w[p, :, 0 + h, :].reshape(-1).astype(np.float64)
            B = w[p, :, 2 + h, :].reshape(-1).astype(np.float64)
            A = w[p, :, 4 + h, :].reshape(-1).astype(np.float64)
            n = counts[c, r]
            seg += -np.sum((A[:n] - B[:n]) / Z[:n])
        total += seg / cnt_c
    return np.float32(total)


def kernel(**inputs) -> np.ndarray:
    from concourse import bass_utils

    in_maps, meta = _host_prep(**inputs)
    cap = meta["cap"]
    patch_tables = bool(int(os.environ.get("KERNEL_PATCH_TABLES", "1")))
    key = (cap, patch_tables)
    if key not in _CACHE:
        _CACHE[key] = _build_program(cap, patch_tables)
    nc = _CACHE[key]

    trace = bool(int(os.environ.get("KERNEL_TRACE", "0")))
    kwargs = {}
    if trace:
        kwargs["tmpdir"] = os.environ.get("KERNEL_TRACE_DIR") or None
    res = bass_utils.run_bass_kernel_spmd(
        nc, in_maps, core_ids=list(range(NCORES)), trace=trace, **kwargs)
    if trace and res.exec_time_ns is not None:
        print(f"HW exec time: {res.exec_time_ns} ns")
    return _gather(res.results, meta)


# revision 41
# speedup vs baseline: 1.0228x; 1.0086x over previous
"""DiffKMeansMultiClass loss on 8 Trainium2 NeuronCores.

Strategy: group samples by class on the host (pure permutation + padding)
so each core computes, per class, distances of its shard to that class's
64 centroids only. Host precomputes normalized xn (fp16) and |xn|^2, so
the device does, per class pair (2 classes packed into 128 PSUM
partitions = 2x64 centroids; class A's slot-j sample occupies partition
rows 0:64 and class B's slot-j sample rows 64:128 via zero-padded
stationary weights):

  d2   = -2 xn.mu + (|xn|^2 + |mu|^2 + BIG*invalid)   PE (2 mu chunks fp16
         + one 2-row mini chunk injecting the additive terms)
  L    = Ln(d2)                                        ACT (fp32)
  s    = Exp(0.5 L) = sqrt(d2)                         ACT (fp16)
  e    = Exp(-3.125 s + 3.125 shift_c)                 ACT (bf16, per-class
         shift keeps the exponent in fp32/bf16 range; legal because the
         softmax ratio v/Z cancels any per-class constant)
  t    = s * e                                         DVE (bf16)
  Z(c) = sum_k e ; B(c) = sum_k c2_k e ; A(c) = sum_k c1_k s e
       = three weighted partition reductions fused into 2 PE matmuls
         (stationary [1_A,1_B,c2_A,c2_B,0,0] over e then
          [0,0,0,0,c1_A,c1_B] over t accumulating into one [6,448] PSUM)
  out  = per-sample (A - B)/Z and the segment mean happen on the host,
         where c1 = -6.25/tau, c2 = ln tau.

No transposes, no DVE reductions: per-sample softmax normalization
reduces over k which lives on PSUM partitions, so the PE's ones/weighted
matmuls do all reductions.
"""

import os
import numpy as np

N, D, C, K = 131072, 256, 20, 64
NCORES = 8
WIN = 448
P = C // 2
DIST_SCALE = 100.0 / 16.0  # 100/sqrt(256) = 6.25
CLUSTER_TEMP = 0.5
SIG_TEMP = 2.0
SIG_MAX = 100.0
RESET_THR = 0.5
BIG = 30000.0  # invalid-centroid d2 penalty (fp16-safe; exp(-3.125*sqrt) = 0)
TEMP = CLUSTER_TEMP * DIST_SCALE  # 3.125

_CACHE = {}


def _build_program(cap, patch_tables=True, use_dr=True):
    import concourse.tile as tile
    from concourse import bacc, mybir

    f32 = mybir.dt.float32
    f16 = mybir.dt.float16
    bf16 = mybir.dt.bfloat16
    f8 = mybir.dt.float8e4
    nw = max(2, -(-cap // 512))
    WIN = cap // nw           # one matmul window (<=512 fp32 PSUM cols)
    assert cap == nw * WIN and WIN <= 512
    nr = C * cap              # rows per core
    cap2 = 2 * cap

    nc = bacc.Bacc("TRN2", target_bir_lowering=False, debug=False)
    xt = nc.dram_tensor("xt", [2, 128, nr], f8, kind="ExternalInput")
    x2r = nc.dram_tensor("x2r", [2, P, cap], f16, kind="ExternalInput")
    wm = nc.dram_tensor("wm", [128, C, 2, 128], f8, kind="ExternalInput")
    wmini = nc.dram_tensor("wmini", [2, P, 128], f16, kind="ExternalInput")
    zst = nc.dram_tensor("zst", [128, P, 2 * nw, 6 * nw], bf16,
                         kind="ExternalInput")
    bias = nc.dram_tensor("bias", [128, P, 2], f32, kind="ExternalInput")
    wout = nc.dram_tensor("wout", [P, 6 * nw, WIN], f32, kind="ExternalOutput")

    Exp = mybir.ActivationFunctionType.Exp
    Ln = mybir.ActivationFunctionType.Ln
    Alu = mybir.AluOpType

    with tile.TileContext(nc) as tc:
        with (
            tc.tile_pool(name="const", bufs=1) as const,
            tc.tile_pool(name="xtp", bufs=4) as xtp,
            tc.tile_pool(name="lp", bufs=2) as lp,
            tc.tile_pool(name="sp", bufs=2) as sp,
            tc.tile_pool(name="ep", bufs=3) as ep,
            tc.tile_pool(name="tp", bufs=3) as tp,
            tc.tile_pool(name="psd", bufs=6, space="PSUM") as psd,
            tc.tile_pool(name="psz", bufs=2, space="PSUM") as psz,
        ):
            wsb = const.tile([128, C, 2, 128], f8)
            nc.sync.dma_start(wsb[:], wm[:])
            msb = const.tile([2, P, 128], f16)
            nc.sync.dma_start(msb[:], wmini[:])
            zsb = const.tile([128, P, 2 * nw, 6 * nw], bf16)
            nc.sync.dma_start(zsb[:], zst[:])
            bsb = const.tile([128, P, 2], f32)
            nc.sync.dma_start(bsb[:], bias[:])

            # tiny dummy activation reading an early const so the 1.3us
            # act-table load (queued just before it, with no waits of its
            # own) runs during the initial DMA fill instead of before the
            # first Ln
            dum2 = const.tile([1, 1], f32)
            nc.scalar.activation(dum2[:], bsb[0:1, 0, 0:1], Exp)

            deferred = []

            for p in range(P):
                cA, cB = 2 * p, 2 * p + 1
                base = cA * cap
                # one D-half per DMA (fully contiguous rows), split across
                # the two hardware DGE queues (SP + gpsimd) in parallel
                xh = xtp.tile([128, 2, cap2], f8, tag="xh")
                nc.sync.dma_start(xh[:, 0, :], xt[0, :, base:base + cap2])
                nc.gpsimd.dma_start(xh[:, 1, :], xt[1, :, base:base + cap2])
                x2t = xtp.tile([2, cap], f16, tag="x2t")
                (nc.gpsimd if p % 2 else nc.sync).dma_start(
                    x2t[:], x2r[:, p, :])

                # window-major so each window's PSUM group closes early and
                # Ln can start while later windows are still on the PE;
                # mu chunks contract all 256 dims in one fp8 DoubleRow pass
                DR = mybir.MatmulPerfMode.DoubleRow
                L = lp.tile([128, cap], f32, tag="L")
                for w in range(nw):
                    ps = psd.tile([128, WIN], f32, tag="ps",
                                  name=f"ps_{p}_{w}")
                    sA = slice(w * WIN, (w + 1) * WIN)
                    sB = slice(cap + w * WIN, cap + (w + 1) * WIN)
                    if use_dr:
                        nc.tensor.matmul(ps[:], wsb[:, cA, :, :],
                                         xh[:, :, sA], perf_mode=DR,
                                         start=True, stop=False)
                        nc.tensor.matmul(ps[:], wsb[:, cB, :, :],
                                         xh[:, :, sB], perf_mode=DR,
                                         start=False, stop=False)
                    else:
                        nc.tensor.matmul(ps[:], wsb[:, cA, 0, :],
                                         xh[:, 0, sA], start=True, stop=False)
                        nc.tensor.matmul(ps[:], wsb[:, cB, 0, :],
                                         xh[:, 0, sB], start=False, stop=False)
                        nc.tensor.matmul(ps[:], wsb[:, cA, 1, :],
                                         xh[:, 1, sA], start=False, stop=False)
                        nc.tensor.matmul(ps[:], wsb[:, cB, 1, :],
                                         xh[:, 1, sB], start=False, stop=False)
                    # rank-2 chunk adds |xn|^2 for both class halves
                    nc.tensor.matmul(ps[:], msb[:, p, :], x2t[:, sA],
                                     start=False, stop=True)
                    # |mu|^2 + invalid penalty enters via the Ln bias
                    nc.scalar.activation(L[:, sA], ps[:], Ln,
                                         bias=bsb[:, p, 0:1])
                s = sp.tile([128, cap], f16, tag="s")
                e = ep.tile([128, cap], bf16, tag="e")
                t = tp.tile([128, cap], bf16, tag="t")
                # window-granular on the last pair to shorten the drain
                for sl in ([slice(0, cap)] if p < P - 1 else
                           [slice(w * WIN, (w + 1) * WIN) for w in range(nw)]):
                    nc.scalar.activation(s[:, sl], L[:, sl], Exp, scale=0.5)
                    nc.scalar.activation(e[:, sl], s[:, sl], Exp,
                                         bias=bsb[:, p, 1:2], scale=-TEMP)
                    nc.vector.tensor_tensor(t[:, sl], s[:, sl], e[:, sl],
                                            op=Alu.mult)

                def zv_stage(p=p, e=e, t=t):
                    # all 2*nw reduction matmuls accumulate into one PSUM
                    # bank: rows 6w..6w+6 belong to window w
                    zps = psz.tile([6 * nw, WIN], f32, tag="zps",
                                   name=f"zps_{p}")
                    nmm = 2 * nw
                    for w in range(nw):
                        sl = slice(w * WIN, (w + 1) * WIN)
                        nc.tensor.matmul(zps[:], zsb[:, p, 2 * w, :], e[:, sl],
                                         start=(w == 0), stop=False)
                        nc.tensor.matmul(zps[:], zsb[:, p, 2 * w + 1, :],
                                         t[:, sl],
                                         start=False, stop=(w == nw - 1))
                    zout = tp.tile([6 * nw, WIN], f32, tag="zout",
                                   name=f"zout_{p}")
                    nc.vector.tensor_copy(zout[:], zps[:])
                    (nc.gpsimd if p % 2 else nc.sync).dma_start(
                        wout[p], zout[:])

                deferred.append(zv_stage)
                # emit reductions two pairs late so the in-order PE queue
                # never waits on ACT/DVE output
                if len(deferred) > 2:
                    deferred.pop(0)()
            while deferred:
                deferred.pop(0)()

    import concourse.bacc as bacc_mod
    from concourse import hw_specs
    orig_tables = hw_specs.get_activation_tables
    want = {Ln, Exp}

    def only_cover(arch):
        full = orig_tables(arch)
        if not any(want <= s for s in full.values()):
            return full
        chosen = next(n for n, s in full.items() if want <= s)
        return {n: (s if n == chosen else set()) for n, s in full.items()}

    if patch_tables:
        bacc_mod.get_activation_tables = only_cover
    try:
        nc.finalize()
    finally:
        bacc_mod.get_activation_tables = orig_tables
    return nc


def _host_prep(data, labels, mu, exp_temp, norm_med, norm_std,
               running_assignment, running_batchsize):
    labels = np.asarray(labels).astype(np.int64)
    data = np.asarray(data, dtype=np.float32)
    mu = np.asarray(mu, dtype=np.float32)

    idx_by_class = [np.flatnonzero(labels == c) for c in range(C)]
    per_core_counts = np.zeros((C, NCORES), dtype=np.int64)
    per_core_idx = [[None] * NCORES for _ in range(C)]
    maxcnt = 1
    for c in range(C):
        splits = np.array_split(idx_by_class[c], NCORES)
        for r in range(NCORES):
            per_core_idx[c][r] = splits[r]
            per_core_counts[c, r] = len(splits[r])
            maxcnt = max(maxcnt, len(splits[r]))

    nw = max(2, -(-maxcnt // 512))
    win = -(-maxcnt // nw)
    win = (win + 7) // 8 * 8  # window multiple of 8
    cap = nw * win
    nr = C * cap

    import ml_dtypes
    f8 = ml_dtypes.float8_e4m3
    a = 1.0 / np.asarray(norm_std, dtype=np.float32)
    b = -np.asarray(norm_med, dtype=np.float32) * a
    xn8 = (data * a[None, :] + b[None, :]).astype(f8)
    x2 = (xn8.astype(np.float32) ** 2).sum(axis=1)  # [N] fp32

    xts, x2rs = [], []
    for r in range(NCORES):
        xc = np.zeros((nr, D), dtype=f8)
        x2c = np.zeros(nr, dtype=np.float32)
        for c in range(C):
            idx = per_core_idx[c][r]
            if len(idx):
                xc[c * cap:c * cap + len(idx)] = xn8[idx]
                x2c[c * cap:c * cap + len(idx)] = x2[idx]
        xts.append(np.ascontiguousarray(xc.T).reshape(2, 128, nr))
        # [2, P, cap]: row 0 = even-class |xn|^2, row 1 = odd-class
        x2rs.append(np.ascontiguousarray(
            x2c.astype(np.float16).reshape(P, 2, cap).transpose(1, 0, 2)))

    mu8 = mu.astype(f8)
    mu8f = mu8.astype(np.float32)
    m2 = (mu8f ** 2).sum(axis=2)  # [C, K]
    thr = np.asarray(running_batchsize, np.float32) / K * RESET_THR
    valid = np.asarray(running_assignment, np.float32) > thr[:, None]
    m2pen = (m2 + BIG * (~valid)).astype(np.float32)

    # zero-padded stationaries: even classes drive PSUM partitions 0:64,
    # odd classes 64:128
    wm = np.zeros((128, C, 2, 128), dtype=f8)
    wmini = np.zeros((2, P, 128), dtype=np.float16)
    for c in range(C):
        h = (c % 2) * K
        wm[:, c, 0, h:h + K] = (-2.0 * mu8f[c, :, :128]).T.astype(f8)
        wm[:, c, 1, h:h + K] = (-2.0 * mu8f[c, :, 128:]).T.astype(f8)
        wmini[c % 2, c // 2, h:h + K] = 1.0

    et = np.asarray(exp_temp, dtype=np.float32)
    tau = 1.0 / (1.0 + np.exp(-et / SIG_TEMP)) * SIG_MAX + 1.0 / SIG_MAX
    c1 = (-DIST_SCALE / tau).astype(np.float32)  # [C, K]
    c2 = np.log(tau).astype(np.float32)

    zst = np.zeros((128, P, 2 * nw, 6 * nw), dtype=ml_dtypes.bfloat16)
    for p in range(P):
        for w in range(nw):
            o = 6 * w
            zst[:K, p, 2 * w, o + 0] = 1.0
            zst[K:, p, 2 * w, o + 1] = 1.0
            zst[:K, p, 2 * w, o + 2] = c2[2 * p]
            zst[K:, p, 2 * w, o + 3] = c2[2 * p + 1]
            zst[:K, p, 2 * w + 1, o + 4] = c1[2 * p]
            zst[K:, p, 2 * w + 1, o + 5] = c1[2 * p + 1]

    shift = np.sqrt(np.median(x2) + np.median(m2, axis=1))  # [C]
    bias = np.zeros((128, P, 2), dtype=np.float32)
    for p in range(P):
        bias[:K, p, 0] = m2pen[2 * p]
        bias[K:, p, 0] = m2pen[2 * p + 1]
        bias[:K, p, 1] = TEMP * shift[2 * p]
        bias[K:, p, 1] = TEMP * shift[2 * p + 1]

    in_maps = [
        {"xt": xts[r], "x2r": x2rs[r], "wm": wm, "wmini": wmini,
         "zst": zst, "bias": bias}
        for r in range(NCORES)
    ]
    meta = {"cap": cap, "nw": nw, "counts": per_core_counts}
    return in_maps, meta


def _gather(results, meta):
    cap = meta["cap"]
    WINR = cap // meta["nw"]
    counts = meta["counts"]  # [C, NCORES]
    total = np.float64(0.0)
    for c in range(C):
        cnt_c = counts[c].sum()
        if cnt_c == 0:
            continue
        p, h = c // 2, c % 2
        seg = np.float64(0.0)
        for r in range(NCORES):
            w = results[r]["wout"].reshape(P, -1, 6, WINR)  # [P, nw, 6, WIN]
            Z = w[p, :, 0 + h, :].reshape(-1).astype(np.float64)
            B = w[p, :, 2 + h, :].reshape(-1).astype(np.float64)
            A = w[p, :, 4 + h, :].reshape(-1).astype(np.float64)
            n = counts[c, r]
            seg += -np.sum((A[:n] - B[:n]) / Z[:n])
        total += seg / cnt_c
    return np.float32(total)


def kernel(**inputs) -> np.ndarray:
    from concourse import bass_utils

    in_maps, meta = _host_prep(**inputs)
    cap = meta["cap"]
    patch_tables = bool(int(os.environ.get("KERNEL_PATCH_TABLES", "1")))
    use_dr = bool(int(os.environ.get("KERNEL_DR", "1")))
    key = (cap, patch_tables, use_dr)
    if key not in _CACHE:
        _CACHE[key] = _build_program(cap, patch_tables, use_dr)
    nc = _CACHE[key]

    trace = bool(int(os.environ.get("KERNEL_TRACE", "0")))
    kwargs = {}
    if trace:
        kwargs["tmpdir"] = os.environ.get("KERNEL_TRACE_DIR") or None
    if (trace or os.environ.get("BASS_TRACE")) and not _CACHE.get(("warm", key)):
        # Warm execution (load NEFF, ramp clocks) before the profiled one;
        # profiling the cold first execution also crashes the runtime.
        os.environ["BASS_NEVER_TRACE"] = "1"
        try:
            bass_utils.run_bass_kernel_spmd(
                nc, in_maps, core_ids=list(range(NCORES)), trace=False)
        finally:
            del os.environ["BASS_NEVER_TRACE"]
        _CACHE[("warm", key)] = True
    res = bass_utils.run_bass_kernel_spmd(
        nc, in_maps, core_ids=list(range(NCORES)), trace=trace, **kwargs)
    if trace and res.exec_time_ns is not None:
        print(f"HW exec time: {res.exec_time_ns} ns")
    return _gather(res.results, meta)


# revision 42
# speedup vs baseline: 1.1150x; 1.0902x over previous
"""DiffKMeansMultiClass loss on 8 Trainium2 NeuronCores.

Strategy: group samples by class on the host (pure permutation + padding)
so each core computes, per class, distances of its shard to that class's
64 centroids only. Host precomputes normalized xn (fp16) and |xn|^2, so
the device does, per class pair (2 classes packed into 128 PSUM
partitions = 2x64 centroids; class A's slot-j sample occupies partition
rows 0:64 and class B's slot-j sample rows 64:128 via zero-padded
stationary weights):

  d2   = -2 xn.mu + (|xn|^2 + |mu|^2 + BIG*invalid)   PE (2 mu chunks fp16
         + one 2-row mini chunk injecting the additive terms)
  L    = Ln(d2)                                        ACT (fp32)
  s    = Exp(0.5 L) = sqrt(d2)                         ACT (fp16)
  e    = Exp(-3.125 s + 3.125 shift_c)                 ACT (bf16, per-class
         shift keeps the exponent in fp32/bf16 range; legal because the
         softmax ratio v/Z cancels any per-class constant)
  t    = s * e                                         DVE (bf16)
  Z(c) = sum_k e ; B(c) = sum_k c2_k e ; A(c) = sum_k c1_k s e
       = three weighted partition reductions fused into 2 PE matmuls
         (stationary [1_A,1_B,c2_A,c2_B,0,0] over e then
          [0,0,0,0,c1_A,c1_B] over t accumulating into one [6,448] PSUM)
  out  = per-sample (A - B)/Z and the segment mean happen on the host,
         where c1 = -6.25/tau, c2 = ln tau.

No transposes, no DVE reductions: per-sample softmax normalization
reduces over k which lives on PSUM partitions, so the PE's ones/weighted
matmuls do all reductions.
"""

import os
import numpy as np

N, D, C, K = 131072, 256, 20, 64
NCORES = 8
WIN = 448
P = C // 2
DIST_SCALE = 100.0 / 16.0  # 100/sqrt(256) = 6.25
CLUSTER_TEMP = 0.5
SIG_TEMP = 2.0
SIG_MAX = 100.0
RESET_THR = 0.5
BIG = 30000.0  # invalid-centroid d2 penalty (fp16-safe; exp(-3.125*sqrt) = 0)
TEMP = CLUSTER_TEMP * DIST_SCALE  # 3.125

_CACHE = {}


def _build_program(cap, patch_tables=True, use_dr=True):
    import concourse.tile as tile
    from concourse import bacc, mybir

    f32 = mybir.dt.float32
    f16 = mybir.dt.float16
    bf16 = mybir.dt.bfloat16
    f8 = mybir.dt.float8e4
    nw = max(2, -(-cap // 512))
    WIN = cap // nw           # one matmul window (<=512 fp32 PSUM cols)
    assert cap == nw * WIN and WIN <= 512
    nr = C * cap              # rows per core
    cap2 = 2 * cap

    nc = bacc.Bacc("TRN2", target_bir_lowering=False, debug=False)
    xt = nc.dram_tensor("xt", [2, 128, nr], f8, kind="ExternalInput")
    x2r = nc.dram_tensor("x2r", [2, P, cap], f16, kind="ExternalInput")
    wm = nc.dram_tensor("wm", [128, C, 2, 128], f8, kind="ExternalInput")
    wmini = nc.dram_tensor("wmini", [2, P, 128], f16, kind="ExternalInput")
    zst = nc.dram_tensor("zst", [128, P, 2 * nw, 6 * nw], bf16,
                         kind="ExternalInput")
    bias = nc.dram_tensor("bias", [128, P, 2], f32, kind="ExternalInput")
    wout = nc.dram_tensor("wout", [P, 6 * nw, WIN], f32, kind="ExternalOutput")

    Exp = mybir.ActivationFunctionType.Exp
    Ln = mybir.ActivationFunctionType.Ln
    Alu = mybir.AluOpType

    with tile.TileContext(nc) as tc:
        with (
            tc.tile_pool(name="const", bufs=1) as const,
            tc.tile_pool(name="xtp", bufs=4) as xtp,
            tc.tile_pool(name="lp", bufs=2) as lp,
            tc.tile_pool(name="sp", bufs=2) as sp,
            tc.tile_pool(name="ep", bufs=3) as ep,
            tc.tile_pool(name="tp", bufs=3) as tp,
            tc.tile_pool(name="psd", bufs=6, space="PSUM") as psd,
            tc.tile_pool(name="psz", bufs=2, space="PSUM") as psz,
        ):
            wsb = const.tile([128, C, 2, 128], f8)
            nc.scalar.dma_start(wsb[:], wm[:])
            msb = const.tile([2, P, 128], f16)
            nc.scalar.dma_start(msb[:], wmini[:])
            zsb = const.tile([128, P, 2 * nw, 6 * nw], bf16)
            nc.scalar.dma_start(zsb[:], zst[:])
            bsb = const.tile([128, P, 2], f32)
            nc.scalar.dma_start(bsb[:], bias[:])

            # tiny dummy activation reading an early const so the 1.3us
            # act-table load (queued just before it, with no waits of its
            # own) runs during the initial DMA fill instead of before the
            # first Ln
            dum2 = const.tile([1, 1], f32)
            nc.scalar.activation(dum2[:], bsb[0:1, 0, 0:1], Exp)

            deferred = []

            for p in range(P):
                cA, cB = 2 * p, 2 * p + 1
                base = cA * cap
                # one D-half per DMA (fully contiguous rows), split across
                # the two hardware DGE queues (SP + gpsimd) in parallel
                xh = xtp.tile([128, 2, cap2], f8, tag="xh")
                nc.sync.dma_start(xh[:, 0, :], xt[0, :, base:base + cap2])
                nc.gpsimd.dma_start(xh[:, 1, :], xt[1, :, base:base + cap2])
                x2t = xtp.tile([2, cap], f16, tag="x2t")
                (nc.gpsimd if p % 2 else nc.sync).dma_start(
                    x2t[:], x2r[:, p, :])

                # window-major so each window's PSUM group closes early and
                # Ln can start while later windows are still on the PE;
                # mu chunks contract all 256 dims in one fp8 DoubleRow pass
                DR = mybir.MatmulPerfMode.DoubleRow
                L = lp.tile([128, cap], f32, tag="L")
                for w in range(nw):
                    ps = psd.tile([128, WIN], f32, tag="ps",
                                  name=f"ps_{p}_{w}")
                    sA = slice(w * WIN, (w + 1) * WIN)
                    sB = slice(cap + w * WIN, cap + (w + 1) * WIN)
                    if use_dr:
                        nc.tensor.matmul(ps[:], wsb[:, cA, :, :],
                                         xh[:, :, sA], perf_mode=DR,
                                         start=True, stop=False)
                        nc.tensor.matmul(ps[:], wsb[:, cB, :, :],
                                         xh[:, :, sB], perf_mode=DR,
                                         start=False, stop=False)
                    else:
                        nc.tensor.matmul(ps[:], wsb[:, cA, 0, :],
                                         xh[:, 0, sA], start=True, stop=False)
                        nc.tensor.matmul(ps[:], wsb[:, cB, 0, :],
                                         xh[:, 0, sB], start=False, stop=False)
                        nc.tensor.matmul(ps[:], wsb[:, cA, 1, :],
                                         xh[:, 1, sA], start=False, stop=False)
                        nc.tensor.matmul(ps[:], wsb[:, cB, 1, :],
                                         xh[:, 1, sB], start=False, stop=False)
                    # rank-2 chunk adds |xn|^2 for both class halves
                    nc.tensor.matmul(ps[:], msb[:, p, :], x2t[:, sA],
                                     start=False, stop=True)
                    # |mu|^2 + invalid penalty enters via the Ln bias
                    nc.scalar.activation(L[:, sA], ps[:], Ln,
                                         bias=bsb[:, p, 0:1])
                s = sp.tile([128, cap], f16, tag="s")
                e = ep.tile([128, cap], bf16, tag="e")
                t = tp.tile([128, cap], bf16, tag="t")
                # window-granular on the last pair to shorten the drain
                for sl in ([slice(0, cap)] if p < P - 1 else
                           [slice(w * WIN, (w + 1) * WIN) for w in range(nw)]):
                    nc.scalar.activation(s[:, sl], L[:, sl], Exp, scale=0.5)
                    nc.scalar.activation(e[:, sl], s[:, sl], Exp,
                                         bias=bsb[:, p, 1:2], scale=-TEMP)
                    nc.vector.tensor_tensor(t[:, sl], s[:, sl], e[:, sl],
                                            op=Alu.mult)

                def zv_stage(p=p, e=e, t=t):
                    # all 2*nw reduction matmuls accumulate into one PSUM
                    # bank: rows 6w..6w+6 belong to window w
                    zps = psz.tile([6 * nw, WIN], f32, tag="zps",
                                   name=f"zps_{p}")
                    nmm = 2 * nw
                    for w in range(nw):
                        sl = slice(w * WIN, (w + 1) * WIN)
                        nc.tensor.matmul(zps[:], zsb[:, p, 2 * w, :], e[:, sl],
                                         start=(w == 0), stop=False)
                        nc.tensor.matmul(zps[:], zsb[:, p, 2 * w + 1, :],
                                         t[:, sl],
                                         start=False, stop=(w == nw - 1))
                    zout = tp.tile([6 * nw, WIN], f32, tag="zout",
                                   name=f"zout_{p}")
                    nc.vector.tensor_copy(zout[:], zps[:])
                    (nc.gpsimd if p % 2 else nc.sync).dma_start(
                        wout[p], zout[:])

                deferred.append(zv_stage)
                # emit reductions two pairs late so the in-order PE queue
                # never waits on ACT/DVE output
                if len(deferred) > 2:
                    deferred.pop(0)()
            while deferred:
                deferred.pop(0)()

    import concourse.bacc as bacc_mod
    from concourse import hw_specs
    orig_tables = hw_specs.get_activation_tables
    want = {Ln, Exp}

    def only_cover(arch):
        full = orig_tables(arch)
        if not any(want <= s for s in full.values()):
            return full
        chosen = next(n for n, s in full.items() if want <= s)
        return {n: (s if n == chosen else set()) for n, s in full.items()}

    if patch_tables:
        bacc_mod.get_activation_tables = only_cover
    try:
        nc.finalize()
    finally:
        bacc_mod.get_activation_tables = orig_tables
    return nc


def _host_prep(data, labels, mu, exp_temp, norm_med, norm_std,
               running_assignment, running_batchsize):
    labels = np.asarray(labels).astype(np.int64)
    data = np.asarray(data, dtype=np.float32)
    mu = np.asarray(mu, dtype=np.float32)

    idx_by_class = [np.flatnonzero(labels == c) for c in range(C)]
    per_core_counts = np.zeros((C, NCORES), dtype=np.int64)
    per_core_idx = [[None] * NCORES for _ in range(C)]
    maxcnt = 1
    for c in range(C):
        splits = np.array_split(idx_by_class[c], NCORES)
        for r in range(NCORES):
            per_core_idx[c][r] = splits[r]
            per_core_counts[c, r] = len(splits[r])
            maxcnt = max(maxcnt, len(splits[r]))

    nw = max(2, -(-maxcnt // 512))
    win = -(-maxcnt // nw)
    win = (win + 7) // 8 * 8  # window multiple of 8
    cap = nw * win
    nr = C * cap

    import ml_dtypes
    f8 = ml_dtypes.float8_e4m3
    a = 1.0 / np.asarray(norm_std, dtype=np.float32)
    b = -np.asarray(norm_med, dtype=np.float32) * a
    xn8 = (data * a[None, :] + b[None, :]).astype(f8)
    x2 = (xn8.astype(np.float32) ** 2).sum(axis=1)  # [N] fp32

    xts, x2rs = [], []
    for r in range(NCORES):
        xc = np.zeros((nr, D), dtype=f8)
        x2c = np.zeros(nr, dtype=np.float32)
        for c in range(C):
            idx = per_core_idx[c][r]
            if len(idx):
                xc[c * cap:c * cap + len(idx)] = xn8[idx]
                x2c[c * cap:c * cap + len(idx)] = x2[idx]
        xts.append(np.ascontiguousarray(xc.T).reshape(2, 128, nr))
        # [2, P, cap]: row 0 = even-class |xn|^2, row 1 = odd-class
        x2rs.append(np.ascontiguousarray(
            x2c.astype(np.float16).reshape(P, 2, cap).transpose(1, 0, 2)))

    mu8 = mu.astype(f8)
    mu8f = mu8.astype(np.float32)
    m2 = (mu8f ** 2).sum(axis=2)  # [C, K]
    thr = np.asarray(running_batchsize, np.float32) / K * RESET_THR
    valid = np.asarray(running_assignment, np.float32) > thr[:, None]
    m2pen = (m2 + BIG * (~valid)).astype(np.float32)

    # zero-padded stationaries: even classes drive PSUM partitions 0:64,
    # odd classes 64:128
    wm = np.zeros((128, C, 2, 128), dtype=f8)
    wmini = np.zeros((2, P, 128), dtype=np.float16)
    for c in range(C):
        h = (c % 2) * K
        wm[:, c, 0, h:h + K] = (-2.0 * mu8f[c, :, :128]).T.astype(f8)
        wm[:, c, 1, h:h + K] = (-2.0 * mu8f[c, :, 128:]).T.astype(f8)
        wmini[c % 2, c // 2, h:h + K] = 1.0

    et = np.asarray(exp_temp, dtype=np.float32)
    tau = 1.0 / (1.0 + np.exp(-et / SIG_TEMP)) * SIG_MAX + 1.0 / SIG_MAX
    c1 = (-DIST_SCALE / tau).astype(np.float32)  # [C, K]
    c2 = np.log(tau).astype(np.float32)

    zst = np.zeros((128, P, 2 * nw, 6 * nw), dtype=ml_dtypes.bfloat16)
    for p in range(P):
        for w in range(nw):
            o = 6 * w
            zst[:K, p, 2 * w, o + 0] = 1.0
            zst[K:, p, 2 * w, o + 1] = 1.0
            zst[:K, p, 2 * w, o + 2] = c2[2 * p]
            zst[K:, p, 2 * w, o + 3] = c2[2 * p + 1]
            zst[:K, p, 2 * w + 1, o + 4] = c1[2 * p]
            zst[K:, p, 2 * w + 1, o + 5] = c1[2 * p + 1]

    shift = np.sqrt(np.median(x2) + np.median(m2, axis=1))  # [C]
    bias = np.zeros((128, P, 2), dtype=np.float32)
    for p in range(P):
        bias[:K, p, 0] = m2pen[2 * p]
        bias[K:, p, 0] = m2pen[2 * p + 1]
        bias[:K, p, 1] = TEMP * shift[2 * p]
        bias[K:, p, 1] = TEMP * shift[2 * p + 1]

    in_maps = [
        {"xt": xts[r], "x2r": x2rs[r], "wm": wm, "wmini": wmini,
         "zst": zst, "bias": bias}
        for r in range(NCORES)
    ]
    meta = {"cap": cap, "nw": nw, "counts": per_core_counts}
    return in_maps, meta


def _gather(results, meta):
    cap = meta["cap"]
    WINR = cap // meta["nw"]
    counts = meta["counts"]  # [C, NCORES]
    total = np.float64(0.0)
    for c in range(C):
        cnt_c = counts[c].sum()
        if cnt_c == 0:
            continue
        p, h = c // 2, c % 2
        seg = np.float64(0.0)
        for r in range(NCORES):
            w = results[r]["wout"].reshape(P, -1, 6, WINR)  # [P, nw, 6, WIN]
            Z = w[p, :, 0 + h, :].reshape(-1).astype(np.float64)
            B = w[p, :, 2 + h, :].reshape(-1).astype(np.float64)
            A = w[p, :, 4 + h, :].reshape(-1).astype(np.float64)
            n = counts[c, r]
            seg += -np.sum((A[:n] - B[:n]) / Z[:n])
        total += seg / cnt_c
    return np.float32(total)


def kernel(**inputs) -> np.ndarray:
    from concourse import bass_utils

    in_maps, meta = _host_prep(**inputs)
    cap = meta["cap"]
    patch_tables = bool(int(os.environ.get("KERNEL_PATCH_TABLES", "1")))
    use_dr = bool(int(os.environ.get("KERNEL_DR", "1")))
    key = (cap, patch_tables, use_dr)
    if key not in _CACHE:
        _CACHE[key] = _build_program(cap, patch_tables, use_dr)
    nc = _CACHE[key]

    trace = bool(int(os.environ.get("KERNEL_TRACE", "0")))
    kwargs = {}
    if trace:
        kwargs["tmpdir"] = os.environ.get("KERNEL_TRACE_DIR") or None
    if (trace or os.environ.get("BASS_TRACE")) and not _CACHE.get(("warm", key)):
        # Warm execution (load NEFF, ramp clocks) before the profiled one;
        # profiling the cold first execution also crashes the runtime.
        os.environ["BASS_NEVER_TRACE"] = "1"
        try:
            bass_utils.run_bass_kernel_spmd(
                nc, in_maps, core_ids=list(range(NCORES)), trace=False)
        finally:
            del os.environ["BASS_NEVER_TRACE"]
        _CACHE[("warm", key)] = True
    res = bass_utils.run_bass_kernel_spmd(
        nc, in_maps, core_ids=list(range(NCORES)), trace=trace, **kwargs)
    if trace and res.exec_time_ns is not None:
        print(f"HW exec time: {res.exec_time_ns} ns")
    return _gather(res.results, meta)
